# revision 2
# baseline (speedup 1.0000x reference)
"""Trainium2 Bass kernel for nn_GAT_1580547974673 (2-layer GAT + pair scoring).

Self-contained: hardcodes all shapes/sharding. Strategy: row-shard the NxN
attention over 8 cores (384 rows each, all 8 heads), pair scoring sharded
over P. Four AllGathers: F2 (tiny, early — lets the DVE z-pipeline start
while the big h gather is in flight), h-aug (bf16), layer-2 [h2|1|f2]
(f32r), and x_out (f32r).

Math restructuring (validated vs reference):
  - f1 = x @ (W @ a1), f2 = x @ (W @ a2)         (weight folding)
  - att_unnorm = exp(lrelu(f1_i + f2_j + M_ij)), M = 0 / -1e9 (mask pre-fold;
    exp of masked entries underflows to exactly 0)
  - rowsum via ones-augmented h in the att @ [h|1] matmul; divide after
  - elu(t) = relu(t) + min(exp(t), 1) - 1        (single Exp, fused combine)
Layout: attention computed transposed (j on partitions, i on free dim).
HW constraints found the hard way: custom DVE / gpsimd partition_broadcast
need base partition 0 (rowsums therefore DMA-bounce from PSUM partition FH),
f32r matmul moving operands need an even free dim, and engine partition
ranges must start 32-aligned. All heavy DMAs are grouped (HWDGE overhead is
per-instruction and serial), pair maps prefetch during layer-1 attention
behind a WAW ordering dep, and the gathered h / exp(z) / pair-map / x_out
tensors are bf16 (halves collective + DMA bytes; PE runs bf16 at full rate).
"""
import numpy as np
from contextlib import ExitStack

import concourse.bass as bass
import concourse.bacc as bacc
import concourse.mybir as mybir
import concourse.tile as tile
import concourse.dve_ops as dve_ops
from concourse.dve_ops import DveOp, OPS
from concourse.dve_spec import Spec, Src0, Src1, C0, C1, One, maxx, minn, relu, lower
from concourse.dve_uop import DveOpSpec
from concourse.bass_utils import run_bass_kernel_spmd
from concourse.masks import make_identity

F32 = mybir.dt.float32
F32R = mybir.dt.float32r
BF16 = mybir.dt.bfloat16
AF = mybir.ActivationFunctionType

# problem shapes (hardcoded per spec)
N, FIN, FH, H, NPAIR = 3072, 512, 64, 8, 2048
NC = 8
IB = N // NC            # 384 rows per core
PB = NPAIR // NC        # 256 pairs per core
NJ = N // 128           # 24 j-blocks
KB = FIN // 128         # 4 k-blocks of the feature dim
SUB = IB // 128         # 3 sub-blocks of the core's row slice
CH = 4                  # j-blocks per exp chunk
NCH = NJ // CH
MASKVAL = -1.0e9
ALPHA = 0.2
HCOLS = H * (FH + 1)    # gathered h-aug columns (per-head [h|1])
H2C = FH + 2            # layer-2 gathered cols: [h2|1|f2]

SIM_NOCOLL = False  # replace collectives with local DMA (for TimelineSim)
DEBUG = False
STOP_AFTER = None   # 'prep' | 'att1' (truncated builds for phase profiling)
MASK_DT = BF16      # mask tile dtype (0/-1e9 are exact in bf16)


def _register_ops():
    """Register the two custom DVE ops (idempotent)."""
    defs = []
    if "GAT_MASK_LRELU" not in dve_ops._SUB_OPCODE_FOR_NAME:
        s = (Src0 + Src1) + C0
        defs.append(DveOp(
            "GAT_MASK_LRELU",
            Spec(body=maxx(s, s * C1),
                 reference=lambda in0, in1, s0, s1, imm2: np.maximum(
                     (in0 + in1) + s0, ((in0 + in1) + s0) * s1)),
            subdim=False, uops_sha={}))
    if "GAT_ELU_COMBINE" not in dve_ops._SUB_OPCODE_FOR_NAME:
        # out = relu(t) + min(E, 1) - 1  with t=Src0, E=Src1(=exp(t))
        defs.append(DveOp(
            "GAT_ELU_COMBINE",
            Spec(body=relu(Src0) + minn(Src1, One) - One,
                 reference=lambda in0, in1, s0, s1, imm2:
                     np.maximum(in0, 0) + np.minimum(in1, 1.0) - 1.0),
            subdim=False, uops_sha={}))
    for op in defs:
        for ver in ("v3", "v4"):
            tmp = DveOpSpec(name=op.name, opcode=0,
                            uops=lower(op.spec, ver=ver), rd1_en=True)
            op.uops_sha[ver] = tmp.sha(ver)
        dve_ops.OPS.append(op)
        dve_ops.CUSTOM_DVE_SPECS[op.name] = op.spec
        dve_ops._SUB_OPCODE_FOR_NAME[op.name] = (
            dve_ops._CUSTOM_DVE_ROW_BASE + len(dve_ops.OPS) - 1)
    ops = {op.name: op for op in dve_ops.OPS}
    return ops["GAT_MASK_LRELU"], ops["GAT_ELU_COMBINE"]


def build(nc, reps=1):
    op_mask_lrelu, op_elu = _register_ops()

    # ---- I/O ----
    xTs_in = nc.dram_tensor("xTs_in", [FIN, IB], F32R, kind="ExternalInput")
    maskT_in = nc.dram_tensor("maskT_in", [N, IB], MASK_DT, kind="ExternalInput")
    W12_in = nc.dram_tensor("W12_in", [FIN, 2 * H], F32R, kind="ExternalInput")
    Wall_in = nc.dram_tensor("Wall_in", [FIN, FIN], F32R, kind="ExternalInput")
    Woh_in = nc.dram_tensor("Woh_in", [64, H * (FH + 2)], F32R, kind="ExternalInput")
    wgt_in = nc.dram_tensor("wgt_in", [FH, FH], F32R, kind="ExternalInput")
    p12T_in = nc.dram_tensor("p12T_in", [N, 2 * PB], BF16, kind="ExternalInput")
    scores_out = nc.dram_tensor("scores_out", [1, PB], F32, kind="ExternalOutput")
    dbg = {}
    if DEBUG:
        for nm, shp in [("d_f1b0", [1, IB]), ("d_f2g", [128, H]),
                        ("d_haug0", [128, HCOLS]), ("d_xct0", [128, IB]),
                        ("d_h2b0", [128, H2C]), ("d_f12b", [1, IB]),
                        ("d_xo0", [128, FH]), ("d_e12", [FH, 2 * PB]),
                        ("d_zt0", [128, IB]), ("d_et0", [128, IB]),
                        ("d_hp0", [FH + 1, IB])]:
            dbg[nm] = nc.dram_tensor(nm, shp, F32, kind="ExternalOutput")

    groups = [list(range(NC))]
    GB = 4  # j-blocks per grouped DMA

    with tile.TileContext(nc) as tc, ExitStack() as octx:
      for rep in range(reps):
        R = f"_r{rep}"
        ctx = ExitStack()
        octx.enter_context(ctx)
        tiny = ctx.enter_context(tc.tile_pool(name="tiny" + R, bufs=1))
        xcp = ctx.enter_context(tc.tile_pool(name="xcp" + R, bufs=1))
        h2pool = ctx.enter_context(tc.tile_pool(name="h2pool" + R, bufs=1))
        npool = ctx.enter_context(tc.tile_pool(name="npool" + R, bufs=2))
        dram = ctx.enter_context(tc.tile_pool(name="dram" + R, bufs=1, space="DRAM"))
        ctx_att1 = ctx.enter_context(ExitStack())
        maskp = ctx_att1.enter_context(tc.tile_pool(name="maskp" + R, bufs=1))
        ztp = ctx_att1.enter_context(tc.tile_pool(name="ztp" + R, bufs=2))
        ep = ctx_att1.enter_context(tc.tile_pool(name="ep" + R, bufs=8))
        ppool = ctx.enter_context(tc.tile_pool(name="ppool" + R, bufs=1))
        ctx_prep = ctx.enter_context(ExitStack())
        fpool = ctx_prep.enter_context(tc.tile_pool(name="fpool" + R, bufs=1))
        hpool = ctx_prep.enter_context(tc.tile_pool(name="hpool" + R, bufs=1))
        ctx_bc = ctx.enter_context(ExitStack())
        cst = ctx_bc.enter_context(tc.tile_pool(name="cst" + R, bufs=1))
        ps_prep = ctx_bc.enter_context(tc.tile_pool(name="ps_prep" + R, bufs=1, space="PSUM"))

        # ---- constant loads (issue order = DMA priority) ----
        xTs = []
        W12s = []
        for kb in range(KB):
            t1 = cst.tile([128, IB], F32R, name=f"xTs{kb}")
            nc.sync.dma_start(t1[:], xTs_in[kb * 128:(kb + 1) * 128, :])
            xTs.append(t1)
        for kb in range(KB):
            t2 = cst.tile([128, 2 * H], F32R, name=f"W12_{kb}")
            nc.sync.dma_start(t2[:], W12_in[kb * 128:(kb + 1) * 128, :])
            W12s.append(t2)
        # mask tiles (resident through both attention layers); only group 0
        # up front — the rest after the F2 chain
        mback = []
        for g4 in range(NJ // GB):
            m = maskp.tile([128, GB, IB], MASK_DT, name=f"maskT{g4}")
            mback.append(m)
        nc.sync.dma_start(
            mback[0][:], maskT_in[0:GB * 128, :].rearrange(
                "(g p) c -> p g c", p=128))
        maskT = [mback[jb // GB][:, jb % GB, :] for jb in range(NJ)]
        Wall = []
        for kb in range(KB):
            t3 = cst.tile([128, FIN], F32R, name=f"Wall{kb}")
            nc.sync.dma_start(t3[:], Wall_in[kb * 128:(kb + 1) * 128, :])
            Wall.append(t3)
        ones8 = tiny.tile([128, H], F32)
        nc.gpsimd.memset(ones8[:], 1.0)
        ones8b = tiny.tile([128, H], BF16)
        nc.gpsimd.memset(ones8b[:], 1.0)
        ones64 = tiny.tile([FH, 1], F32)
        nc.gpsimd.memset(ones64[:], 1.0)
        ident = tiny.tile([128, 128], F32)
        make_identity(nc, ident[:])

        # ---- stage B: local F2 first (gates the tiny early gather) ----
        F2st = fpool.tile([128, SUB, H], F32, name="F2st")
        for s in range(SUB):
            ps_f2 = ps_prep.tile([128, H], F32, tag="pf2", name=f"ps_f2{s}")
            for kb in range(KB):
                nc.tensor.matmul(ps_f2[:], xTs[kb][:, s * 128:(s + 1) * 128],
                                 W12s[kb][:, H:2 * H],
                                 start=(kb == 0), stop=(kb == KB - 1))
            nc.vector.tensor_copy(F2st[:, s, :], ps_f2[:])
        f2loc_d = dram.tile([IB, H], F32)
        nc.sync.dma_start(
            f2loc_d[:].rearrange("(s p) c -> p s c", p=128), F2st[:])
        f2g_d = dram.tile([N, H], F32, addr_space="Shared")
        if SIM_NOCOLL:
            for c in range(1):
                nc.sync.dma_start(f2g_d[c * IB:(c + 1) * IB, :], f2loc_d[:])
        else:
            nc.gpsimd.collective_compute(
                "AllGather", mybir.AluOpType.bypass, replica_groups=groups,
                ins=[f2loc_d[:].opt()], outs=[f2g_d[:].opt()])
        F2g = fpool.tile([128, NJ, H], F32, name="F2g")
        nc.sync.dma_start(F2g[:], f2g_d[:].rearrange("(g p) c -> p g c", p=128))
        # later mask groups: issued after the latency-critical F2g readback,
        # with a tiny WAW dep so their transfers queue behind it
        for g4 in range(1, NJ // GB):
            nc.vector.tensor_copy(mback[g4][0:1, 0, 0:2], F2g[0:1, 0, 0:2])
            nc.sync.dma_start(
                mback[g4][:],
                maskT_in[g4 * GB * 128:(g4 + 1) * GB * 128, :].rearrange(
                    "(g p) c -> p g c", p=128))

        # f1 rows (each at partition 0), psum->sbuf copies on the idle DVE
        f1b = []
        for h in range(H):
            ps_f1 = ps_prep.tile([1, IB], F32, tag="pf1", bufs=2, name=f"ps_f1{h}")
            for kb in range(KB):
                nc.tensor.matmul(ps_f1[:], W12s[kb][:, h:h + 1], xTs[kb][:],
                                 start=(kb == 0), stop=(kb == KB - 1))
            row = fpool.tile([1, IB], F32, name=f"f1row{h}")
            nc.vector.tensor_copy(row[:], ps_f1[:])
            t = fpool.tile([128, IB], F32, name=f"f1b{h}")
            nc.gpsimd.partition_broadcast(t[:], row[:])
            f1b.append(t)
        if DEBUG:
            nc.sync.dma_start(dbg["d_f1b0"][:], f1b[0][0:1, :])

        # remaining constants (issued after the latency-critical F2 chain)
        Woh = tiny.tile([64, H, FH + 2], F32R, name="Woh")
        nc.sync.dma_start(Woh[:], Woh_in[:].rearrange("p (h c) -> p h c", h=H))
        wgt = tiny.tile([FH, FH], F32R)
        nc.sync.dma_start(wgt[:], wgt_in[:])

        # ---- stage C: local h (f32r) -> bf16 aug layout -> gather ----
        hloc_d = dram.tile([IB, HCOLS], BF16)
        for s in range(SUB):
            ps_h = ps_prep.tile([128, FIN], F32, tag="ph", bufs=2, name=f"ps_h{s}")
            for kb in range(KB):
                nc.tensor.matmul(ps_h[:],
                                 xTs[kb][:, s * 128:(s + 1) * 128],
                                 Wall[kb][:],
                                 start=(kb == 0), stop=(kb == KB - 1))
            hsb = fpool.tile([128, HCOLS], BF16, name=f"hloc{s}")
            hsb3 = hsb[:].rearrange("p (h f) -> p h f", h=H)
            nc.scalar.copy(hsb3[:, :, 0:FH],
                           ps_h[:].rearrange("p (h f) -> p h f", h=H))
            nc.scalar.copy(hsb3[:, :, FH], ones8b[:])
            nc.sync.dma_start(hloc_d[s * 128:(s + 1) * 128, :], hsb[:])
        hg_d = dram.tile([N, HCOLS], BF16, addr_space="Shared")
        if SIM_NOCOLL:
            for c in range(1):
                nc.sync.dma_start(hg_d[c * IB:(c + 1) * IB, :], hloc_d[:])
        else:
            nc.gpsimd.collective_compute(
                "AllGather", mybir.AluOpType.bypass, replica_groups=groups,
                ins=[hloc_d[:].opt()], outs=[hg_d[:].opt()])

        hback = []
        for g4 in range(NJ // GB):
            ht = hpool.tile([128, GB, HCOLS], BF16, name=f"hback{g4}")
            if SIM_NOCOLL and g4 > 0:
                nc.vector.tensor_copy(ht[0:1, 0, 0:2], hback[0][0:1, 0, 0:2])
            nc.sync.dma_start(
                ht[:], hg_d[g4 * GB * 128:(g4 + 1) * GB * 128, :].rearrange(
                    "(g p) c -> p g c", p=128))
            hback.append(ht)
        haug_r = [hback[jb // GB][:, jb % GB, :] for jb in range(NJ)]

        # pair maps (bf16): prefetched during layer-1 attention, ordered
        # behind the latency-critical h readbacks via a tiny WAW dep
        p12r = []
        for g4 in range(NJ // GB):
            tp = ppool.tile([128, GB, 2 * PB], BF16, name=f"p12_{g4}")
            nc.vector.tensor_copy(tp[0:1, 0, 0:2], hback[-1][0:1, 0, 0:2])
            nc.sync.dma_start(
                tp[:], p12T_in[g4 * GB * 128:(g4 + 1) * GB * 128, :].rearrange(
                    "(g p) c -> p g c", p=128))
            p12r.append(tp)
        p12c = [p12r[jb // GB][:, jb % GB, :] for jb in range(NJ)]
        if DEBUG:
            nc.sync.dma_start(dbg["d_f2g"][:], F2g[:, 0, :])
            dh0 = hpool.tile([128, HCOLS], F32, name="dh0")
            nc.vector.tensor_copy(dh0[:], haug_r[0])
            nc.sync.dma_start(dbg["d_haug0"][:], dh0[:])

        ctx_bc.close()
        ps_small = ctx.enter_context(tc.tile_pool(name="ps_small" + R, bufs=1, space="PSUM"))
        ps_hp_pool = ctx.enter_context(tc.tile_pool(name="ps_hp" + R, bufs=2, space="PSUM"))
        ps_l2 = ctx.enter_context(tc.tile_pool(name="ps_l2" + R, bufs=1, space="PSUM"))

        if STOP_AFTER == "prep":
            srow0 = npool.tile([1, PB], F32)
            nc.gpsimd.memset(srow0[:], 0.0)
            nc.vector.tensor_copy(srow0[:, 0:H], F2g[0:1, 0, :])
            nc.vector.tensor_copy(srow0[:, H:H + HCOLS // 8],
                                  haug_r[0][0:1, 0:HCOLS:8].bitcast(BF16))
            nc.sync.dma_start(scores_out[:], srow0[:])
            ctx.close()
            continue

        # ---- stage D: layer-1 attention, per head ----
        elup = [xcp.tile([FH, IB], F32R, name=f"elup{h}") for h in range(H)]
        ps_h2 = [ps_l2.tile([128, FH + 2], F32, name=f"ps_h2_{s}")
                 for s in range(SUB)]
        ps_f12 = ps_l2.tile([1, IB], F32, name="ps_f12")

        def attention(head, haug_col0, f2col_of, f1b_t, et_dt, out_sb):
            """One attention unit: out_sb[:] = elu(att @ h)."""
            ps_hp = ps_hp_pool.tile([FH + 1, IB], F32, tag="hp",
                                    name=f"ps_hp{head}")
            for c in range(NCH):
                zt = ztp.tile([128, CH, IB], F32, tag="zt", name=f"zt{head}_{c}")
                for g in range(CH):
                    jb = c * CH + g
                    nc.vector._custom_dve(
                        op_mask_lrelu, out=zt[:, g, :], in0=f1b_t[:],
                        in1=maskT[jb], s0=f2col_of(jb), s1=ALPHA)
                et = ep.tile([128, CH, IB], et_dt, tag="et", bufs=8,
                             name=f"et{head}_{c}")
                nc.scalar.activation(et[:], zt[:], AF.Exp)
                if DEBUG and head == 0 and c == 0:
                    nc.sync.dma_start(dbg["d_zt0"][:], zt[:, 0, :])
                    det = npool.tile([128, IB], F32, tag="dbg2", bufs=1, name="det")
                    nc.vector.tensor_copy(det[:], et[:, 0, :])
                    nc.sync.dma_start(dbg["d_et0"][:], det[:])
                for g in range(CH):
                    jb = c * CH + g
                    nc.tensor.matmul(
                        ps_hp[:], haug_r[jb][:, haug_col0:haug_col0 + FH + 1],
                        et[:, g, :], start=(jb == 0), stop=(jb == NJ - 1))
            # normalize + elu; rowsum lives at PSUM partition FH -> DMA-bounce
            # to partition 0 (the only partition-moving engine is DMA; gpsimd
            # broadcast + custom DVE ops require base partition 0 on HW)
            rs64 = npool.tile([128, IB], F32, bufs=2, tag="rs64", name=f"rs64{head}")
            nc.scalar.copy(rs64[FH:FH + 1, :], ps_hp[FH:FH + 1, :])
            rsum = npool.tile([1, IB], F32, bufs=2, tag="rsum", name=f"rsum{head}")
            nc.sync.dma_start(rsum[:], rs64[FH:FH + 1, :])
            rrow = npool.tile([1, IB], F32, bufs=2, tag="rrow", name=f"rrow{head}")
            nc.vector.reciprocal_approx_fast(rrow[:], rsum[:])
            rb = npool.tile([FH, IB], F32, tag="rb", name=f"rb{head}")
            nc.gpsimd.partition_broadcast(rb[:], rrow[:])
            hp_sb = npool.tile([FH, IB], F32, tag="hps", name=f"hps{head}")
            nc.scalar.copy(hp_sb[:], ps_hp[0:FH, :])
            t_n = npool.tile([FH, IB], F32, tag="tn", name=f"tn{head}")
            nc.gpsimd.tensor_mul(t_n[:], hp_sb[:], rb[:])
            e_n = npool.tile([FH, IB], F32, tag="en", name=f"en{head}")
            nc.scalar.activation(e_n[:], t_n[:], AF.Exp)
            nc.vector._custom_dve(op_elu, out=out_sb[:], in0=t_n[:], in1=e_n[:])
            if DEBUG and head == 0:
                dhp = npool.tile([FH + 1, IB], F32, tag="dbg1", bufs=1, name="dhp")
                nc.scalar.copy(dhp[:], ps_hp[:])
                nc.sync.dma_start(dbg["d_hp0"][:], dhp[:])

        for head in range(H):
            attention(head, head * (FH + 1),
                      lambda jb, h=head: F2g[:, jb, h:h + 1],
                      f1b[head], BF16, elup[head][:])
            # layer-2 h / f1 accumulation as soon as this head's xc is final
            for s in range(SUB):
                nc.tensor.matmul(ps_h2[s][:],
                                 elup[head][:, s * 128:(s + 1) * 128],
                                 Woh[:, head, :],
                                 start=(head == 0), stop=(head == H - 1))
            nc.tensor.matmul(ps_f12[:], Woh[:, head, FH + 1:FH + 2],
                             elup[head][:], start=(head == 0), stop=(head == H - 1))

        if STOP_AFTER == "att1":
            srow0 = npool.tile([1, PB], F32)
            nc.vector.tensor_copy(srow0[:], elup[0][0:1, 0:PB].bitcast(F32))
            nc.sync.dma_start(scores_out[:], srow0[:])
            ctx.close()
            continue
        ctx_prep.close()
        xopool = ctx.enter_context(tc.tile_pool(name="xopool" + R, bufs=1))
        epool = ctx.enter_context(tc.tile_pool(name="epool" + R, bufs=1))

        # ---- stage E: finish layer-2 h -> gather [h2|1|f2] ----
        h2st = h2pool.tile([128, SUB, H2C], F32R, name="h2st")
        for s in range(SUB):
            nc.scalar.copy(h2st[:, s, 0:FH], ps_h2[s][:, 0:FH])
            nc.scalar.copy(h2st[:, s, FH:FH + 1], ones8[:, 0:1])
            nc.scalar.copy(h2st[:, s, FH + 1:FH + 2], ps_h2[s][:, FH:FH + 1])
        f12row = h2pool.tile([1, IB], F32)
        nc.scalar.copy(f12row[:], ps_f12[:])
        f12b = h2pool.tile([128, IB], F32)
        nc.gpsimd.partition_broadcast(f12b[:], f12row[:])

        h2loc_d = dram.tile([IB, H2C], F32R)
        nc.sync.dma_start(
            h2loc_d[:].rearrange("(s p) c -> p s c", p=128), h2st[:])
        h2g_d = dram.tile([N, H2C], F32R, addr_space="Shared")
        if SIM_NOCOLL:
            for c in range(1):
                nc.sync.dma_start(h2g_d[c * IB:(c + 1) * IB, :], h2loc_d[:])
        else:
            nc.gpsimd.collective_compute(
                "AllGather", mybir.AluOpType.bypass, replica_groups=groups,
                ins=[h2loc_d[:].opt()], outs=[h2g_d[:].opt()])
        h2back = []
        for g4 in range(NJ // GB):
            t = h2pool.tile([128, GB, H2C], F32R, name=f"h2b{g4}")
            if SIM_NOCOLL and g4 > 0:
                nc.vector.tensor_copy(t[0:1, 0, 0:2], h2back[0][0:1, 0, 0:2])
            nc.sync.dma_start(
                t[:], h2g_d[g4 * GB * 128:(g4 + 1) * GB * 128, :].rearrange(
                    "(g p) c -> p g c", p=128))
            h2back.append(t)
        h2r = [h2back[jb // GB][:, jb % GB, 0:FH + 1] for jb in range(NJ)]

        if DEBUG:
            dh2 = h2pool.tile([128, H2C], F32, name="dh2")
            nc.vector.tensor_copy(dh2[:], h2back[0][:, 0, :].bitcast(F32))
            nc.sync.dma_start(dbg["d_h2b0"][:], dh2[:])
            nc.sync.dma_start(dbg["d_f12b"][:], f12b[0:1, :])
            nc.sync.dma_start(dbg["d_xct0"][0:FH, :], elup[0][:].bitcast(F32))
            nc.sync.dma_start(dbg["d_xct0"][FH:2 * FH, :], elup[1][:].bitcast(F32))

        # ---- stage F: layer-2 attention (single head) ----
        xoT = h2pool.tile([FH, IB], F32)
        ps_hp2 = ps_hp_pool.tile([FH + 1, IB], F32, tag="hp", name="ps_hp2")
        for c in range(NCH):
            zt = ztp.tile([128, CH, IB], F32, tag="zt", name=f"zt2_{c}")
            for g in range(CH):
                jb = c * CH + g
                nc.vector._custom_dve(
                    op_mask_lrelu, out=zt[:, g, :], in0=f12b[:],
                    in1=maskT[jb],
                    s0=h2back[jb // GB][:, jb % GB, FH + 1:FH + 2].bitcast(F32),
                    s1=ALPHA)
            et = ep.tile([128, CH, IB], F32R, tag="et2", bufs=2, name=f"et2_{c}")
            nc.scalar.activation(et[:], zt[:], AF.Exp)
            for g in range(CH):
                jb = c * CH + g
                nc.tensor.matmul(ps_hp2[:], h2r[jb], et[:, g, :],
                                 start=(jb == 0), stop=(jb == NJ - 1))
        rs64b = npool.tile([128, IB], F32, bufs=2, tag="rs64", name="rs64b")
        nc.scalar.copy(rs64b[FH:FH + 1, :], ps_hp2[FH:FH + 1, :])
        rsum2 = npool.tile([1, IB], F32, bufs=2, tag="rsum", name="rsum2")
        nc.sync.dma_start(rsum2[:], rs64b[FH:FH + 1, :])
        rrow2 = npool.tile([1, IB], F32, bufs=2, tag="rrow", name="rrow2")
        nc.vector.reciprocal_approx_fast(rrow2[:], rsum2[:])
        rb2 = npool.tile([FH, IB], F32, tag="rb", name="rb2")
        nc.gpsimd.partition_broadcast(rb2[:], rrow2[:])
        hp_sb2 = npool.tile([FH, IB], F32, tag="hps", name="hps2")
        nc.scalar.copy(hp_sb2[:], ps_hp2[0:FH, :])
        t_n2 = npool.tile([FH, IB], F32, tag="tn", name="tn2")
        nc.gpsimd.tensor_mul(t_n2[:], hp_sb2[:], rb2[:])
        e_n2 = npool.tile([FH, IB], F32, tag="en", name="en2")
        nc.scalar.activation(e_n2[:], t_n2[:], AF.Exp)
        nc.vector._custom_dve(op_elu, out=xoT[:], in0=t_n2[:], in1=e_n2[:])

        # ---- stage G: x_out natural layout + gather ----
        xonat = xopool.tile([128, SUB, FH], BF16, name="xonat")
        for s in range(SUB):
            ps_tr = ps_small.tile([128, FH], F32, tag="pss")
            nc.tensor.transpose(ps_tr[:], xoT[:, s * 128:(s + 1) * 128],
                                ident[0:FH, 0:FH])
            nc.scalar.copy(xonat[:, s, :], ps_tr[:])
        xoloc_d = dram.tile([IB, FH], BF16)
        nc.sync.dma_start(
            xoloc_d[:].rearrange("(s p) c -> p s c", p=128), xonat[:])
        xog_d = dram.tile([N, FH], BF16, addr_space="Shared")
        if SIM_NOCOLL:
            for c in range(1):
                nc.sync.dma_start(xog_d[c * IB:(c + 1) * IB, :], xoloc_d[:])
        else:
            nc.gpsimd.collective_compute(
                "AllGather", mybir.AluOpType.bypass, replica_groups=groups,
                ins=[xoloc_d[:].opt()], outs=[xog_d[:].opt()])
        xor_ = []
        for g4 in range(NJ // GB):
            r = xopool.tile([128, GB, FH], BF16, name=f"xor{g4}")
            if SIM_NOCOLL and g4 > 0:
                nc.vector.tensor_copy(r[0:1, 0, 0:2], xor_[0][0:1, 0, 0:2])
            nc.sync.dma_start(
                r[:], xog_d[g4 * GB * 128:(g4 + 1) * GB * 128, :].rearrange(
                    "(g p) c -> p g c", p=128))
            xor_.append(r)
        xorc = [xor_[jb // GB][:, jb % GB, :] for jb in range(NJ)]
        if DEBUG:
            dxo = xopool.tile([128, FH], F32, name="dxo")
            nc.vector.tensor_copy(dxo[:], xorc[0])
            nc.sync.dma_start(dbg["d_xo0"][:], dxo[:])

        # ---- stage H: pair embeddings + scores ----
        ps_e12 = ps_small.tile([FH, 2 * PB], F32, tag="pse", bufs=1, name="ps_e12")
        for jb in range(NJ):
            nc.tensor.matmul(ps_e12[:], xorc[jb], p12c[jb],
                             start=(jb == 0), stop=(jb == NJ - 1))
        e12sb = epool.tile([FH, 2 * PB], F32R)
        nc.scalar.copy(e12sb[:], ps_e12[:])
        if DEBUG:
            nc.sync.dma_start(dbg["d_e12"][:], e12sb[:].bitcast(F32))

        ps_g = ps_small.tile([FH, PB], F32, tag="pss", name="ps_g")
        nc.tensor.matmul(ps_g[:], wgt[:],
                         e12sb[:, 0:PB], start=True, stop=True)
        prod = epool.tile([FH, PB], F32)
        nc.vector.tensor_mul(prod[:], ps_g[:], e12sb[:, PB:2 * PB].bitcast(F32))
        ps_s = ps_small.tile([1, PB], F32, tag="pss", name="ps_s")
        nc.tensor.matmul(ps_s[:], ones64[:], prod[:], start=True, stop=True)
        srow = epool.tile([1, PB], F32)
        nc.scalar.copy(srow[:], ps_s[:])
        nc.sync.dma_start(scores_out[:], srow[:])
        ctx.close()

    return nc


_CACHE = {}


def _get_nc(reps=1):
    key = f"nc{reps}"
    if key not in _CACHE:
        nc = bacc.Bacc(None, target_bir_lowering=False, debug=False, num_devices=NC)
        build(nc, reps=reps)
        nc.compile()
        _CACHE[key] = nc
    return _CACHE[key]


def prep_inputs(x, adj, pair1_map, pair2_map, Wh, a1h, a2h, W_out, a1_out,
                a2_out, weight):
    import ml_dtypes
    x = np.ascontiguousarray(np.asarray(x, np.float32))
    adj = np.asarray(adj)
    maskT = np.where(adj > 0, np.float32(0.0), np.float32(MASKVAL)).T  # [j, i]
    maskT = np.ascontiguousarray(maskT).astype(ml_dtypes.bfloat16)
    xT = np.ascontiguousarray(x.T)                                     # [FIN, N]
    Wall = np.ascontiguousarray(
        np.transpose(np.asarray(Wh, np.float64), (1, 0, 2)).reshape(FIN, H * FH)
    ).astype(np.float32)
    w1 = np.einsum("hkf,hf->kh", np.asarray(Wh, np.float64), np.asarray(a1h, np.float64))
    w2 = np.einsum("hkf,hf->kh", np.asarray(Wh, np.float64), np.asarray(a2h, np.float64))
    W12 = np.concatenate([w1, w2], axis=1).astype(np.float32)          # [FIN, 16]
    w1o = np.asarray(W_out, np.float64) @ np.asarray(a1_out, np.float64)
    w2o = np.asarray(W_out, np.float64) @ np.asarray(a2_out, np.float64)
    Wo = np.concatenate([np.asarray(W_out, np.float64), w2o[:, None],
                         w1o[:, None]], axis=1)                        # [FIN, 66]
    Woh = np.ascontiguousarray(
        Wo.reshape(H, FH, FH + 2).transpose(1, 0, 2).reshape(FH, H * (FH + 2))
    ).astype(np.float32)                                               # [64, 8*66]
    p1T = np.ascontiguousarray(np.asarray(pair1_map, np.float32).T)    # [N, NPAIR]
    p2T = np.ascontiguousarray(np.asarray(pair2_map, np.float32).T)
    wgt = np.ascontiguousarray(np.asarray(weight, np.float32))

    in_maps = []
    for c in range(NC):
        i0, i1 = c * IB, (c + 1) * IB
        p0, p1 = c * PB, (c + 1) * PB
        in_maps.append({
            "xTs_in": np.ascontiguousarray(xT[:, i0:i1]),
            "maskT_in": np.ascontiguousarray(maskT[:, i0:i1]),
            "W12_in": W12,
            "Wall_in": Wall,
            "Woh_in": Woh,
            "wgt_in": wgt,
            "p12T_in": np.ascontiguousarray(
                np.concatenate([p1T[:, p0:p1], p2T[:, p0:p1]],
                               axis=1)).astype(ml_dtypes.bfloat16),
        })
    return in_maps


def run(inputs, trace=False, **kw):
    nc = _get_nc()
    in_maps = prep_inputs(**inputs)
    res = run_bass_kernel_spmd(nc, in_maps, list(range(NC)), trace=trace, **kw)
    scores = np.concatenate(
        [res.results[c]["scores_out"].reshape(-1) for c in range(NC)])
    return scores.astype(np.float32), res


def kernel(**inputs):
    return run(inputs)[0]


def _make_fn(nc, in_maps):
    import jax
    from jax.sharding import Mesh, PartitionSpec, NamedSharding
    from jax.experimental.shard_map import shard_map
    from concourse import bass2jax
    import concourse.mybir as _mb

    bass2jax.install_neuronx_cc_hook()
    partition_name = nc.partition_id_tensor.name if nc.partition_id_tensor else None
    in_names, out_names, out_avals, zero_outs = [], [], [], []
    for alloc in nc.m.functions[0].allocations:
        if not isinstance(alloc, _mb.MemoryLocationSet):
            continue
        name = alloc.memorylocations[0].name
        if alloc.kind == "ExternalInput":
            if name != partition_name:
                in_names.append(name)
        elif alloc.kind == "ExternalOutput":
            shape = list(alloc.tensor_shape)
            npdt = _mb.dt.np(alloc.dtype)
            out_names.append(name)
            out_avals.append(jax.core.ShapedArray(shape, npdt))
            zero_outs.append(np.zeros(shape, npdt))
    n_params = len(in_names)
    n_outs = len(out_names)
    all_in_names = list(in_names) + list(out_names)
    if partition_name is not None:
        all_in_names.append(partition_name)

    def _body(*args):
        operands = list(args)
        if partition_name is not None:
            operands.append(bass2jax.partition_id_tensor())
        outs = bass2jax._bass_exec_p.bind(
            *operands, out_avals=tuple(out_avals), in_names=tuple(all_in_names),
            out_names=tuple(out_names), lowering_input_output_aliases=(),
            sim_require_finite=True, sim_require_nnan=True, nc=nc)
        return tuple(outs)

    devices = jax.devices()[:NC]
    mesh = Mesh(np.asarray(devices), ("core",))
    in_specs = (PartitionSpec("core"),) * (n_params + n_outs)
    out_specs = (PartitionSpec("core"),) * n_outs
    fn = jax.jit(shard_map(_body, mesh=mesh, in_specs=in_specs,
                           out_specs=out_specs, check_rep=False),
                 keep_unused=True)
    concat_in = [
        np.concatenate([np.asarray(in_maps[c][nm]) for c in range(NC)], axis=0)
        for nm in in_names]
    concat_zeros = [np.zeros((NC * z.shape[0], *z.shape[1:]), z.dtype)
                    for z in zero_outs]
    sh = NamedSharding(mesh, PartitionSpec("core"))
    dev_in = [jax.device_put(a, sh) for a in concat_in]
    dev_zero = [jax.device_put(a, sh) for a in concat_zeros]
    return fn, dev_in, dev_zero


def bench(inputs, iters=6, kreps=5):
    """Device time per kernel pass, via the in-NEFF replication slope."""
    import time
    import jax
    in_maps = prep_inputs(**inputs)
    fns = {}
    for reps in (1, kreps):
        nc = _get_nc(reps=reps)
        fn, dev_in, dev_zero = _make_fn(nc, in_maps)
        jax.block_until_ready(fn(*dev_in, *dev_zero))  # warm/compile
        fns[reps] = (fn, dev_in, dev_zero)

    def once(reps):
        fn, dev_in, dev_zero = fns[reps]
        t0 = time.perf_counter()
        jax.block_until_ready(fn(*dev_in, *dev_zero))
        return time.perf_counter() - t0

    t1s, tks, diffs = [], [], []
    for _ in range(3 * iters):
        a = once(1)
        b = once(kreps)
        c = once(1)
        t1s += [a, c]
        tks.append(b)
        diffs.append(b - (a + c) / 2)
    diffs.sort()
    med = diffs[len(diffs) // 2]
    out = {
        "t1_ns": min(t1s) * 1e9,
        f"t{kreps}_ns": min(tks) * 1e9,
        "pooled_med_ns": med / (kreps - 1) * 1e9,
        "per_exec_ns": max(med / (kreps - 1) * 1e9, 0.0),
    }
    return out


if __name__ == "__main__":
    rng = np.random.default_rng(0)
    ins = dict(
        x=rng.standard_normal((N, FIN), dtype=np.float32),
        adj=(rng.random((N, N)) < 0.5).astype(np.int32),
        pair1_map=rng.standard_normal((NPAIR, N), dtype=np.float32),
        pair2_map=rng.standard_normal((NPAIR, N), dtype=np.float32),
        Wh=rng.standard_normal((H, FIN, FH), dtype=np.float32) * 0.1,
        a1h=rng.standard_normal((H, FH), dtype=np.float32) * 0.3,
        a2h=rng.standard_normal((H, FH), dtype=np.float32) * 0.3,
        W_out=rng.standard_normal((FIN, FH), dtype=np.float32) * 0.1,
        a1_out=rng.standard_normal((FH,), dtype=np.float32) * 0.3,
        a2_out=rng.standard_normal((FH,), dtype=np.float32) * 0.3,
        weight=rng.standard_normal((FH, FH), dtype=np.float32) * 0.1,
    )
    out = kernel(**ins)
    print("scores:", out.shape, out[:8])
